# revision 1
# baseline (speedup 1.0000x reference)
"""Contrastive loss kernel for Trainium2, 8 NeuronCores (SPMD).

Math (matches the reference):
    z = concat(normalize(z_i), normalize(z_j))        # (2B, D) = (8192, 256)
    sim = (z @ z.T) / T
    positives[g] = sim[g, (g+B) mod 2B]               # (2B,)
    neg_max[g] = max_{j != g} sim[g, j]
    loss = mean(neg_max) - logsumexp(positives)       # scalar

Sharding: data-parallel over rows. Core k receives z rolled by -1024*k so its
band is always rows [0, 1024) of its local copy -> identical static program on
every core.

v6 design (normalize-late, host norms):
  The device computes the RAW Gram matrix G = z @ z.T in bf16 and applies only
  the column normalization 1/||z_j|| during PSUM evacuation; the row factor
  1/||z_i|| is monotone w.r.t. the row max, so it moves to the host (f64).
  Row norms are O(N*D) input preprocessing, so the host computes them in f64
  (alongside the np.roll staging) and ships inv as a tiny input tensor.

  The bf16 transposed operand zT is produced purely by DMA (gpsimd cast-DMA
  f32->bf16, store, xbar transpose-load) with no compute engines on that
  path, so matmul waves start as soon as the first band lands (~7us).

  Cell structure: stationary operand = 128-column j-chunk, moving operand =
  the core's own 1024 rows -> psum [128 j, 1024 i]. With j on partitions, the
  column scale inv[j] is a per-partition AP that ACT's activation fuses into
  the PSUM->SBUF copy for free (a few cells evacuate on DVE to balance).
  DVE max-accumulates each cell into acc [128, 1024]. Host: final 128-way
  max, exact norm application, mean/LSE in f64.
"""

import numpy as np

TEMPERATURE = 0.1
B, D = 4096, 256
R = 2 * B                # 8192 total rows
NCORES = 8
MROWS = R // NCORES      # 1024 rows per core
P = 128                  # SBUF partitions
NT_ROW = R // P          # 64 row tiles of (128, 256)
MB = MROWS // P          # 8 blocks of own rows
CH = 8                   # chunks (1024 rows each)
TPG = NT_ROW // CH       # 8 row tiles per chunk
KC = D // P              # 2 contraction chunks of 128
NC_CELL = R // P         # 64 cells (j-chunks of 128)
BIG = 30000.0            # diag mask subtrahend
# cells whose evacuation runs on DVE instead of ACT (load balance knob)
DVE_CELLS = frozenset((5, 11, 17, 23, 29, 35, 41, 47, 53, 59))

_CACHE = {}


def _host_constants():
    ident = np.eye(P, dtype=np.float32)
    bigI = (np.eye(P) * BIG).astype(np.float32)
    return {"ident_f": ident, "bigI": bigI}


def _build_nc():
    from contextlib import ExitStack

    import concourse.bass as bass
    import concourse.mybir as mybir
    import concourse.tile as tile
    from concourse import bacc

    f32 = mybir.dt.float32
    bf16 = mybir.dt.bfloat16
    X = mybir.AxisListType.X

    nc = bacc.Bacc(
        "TRN2",
        target_bir_lowering=False,
        debug=False,
        enable_asserts=False,
        num_devices=NCORES,
    )

    z_dram = nc.dram_tensor("z", [R, D], f32, kind="ExternalInput")
    inv_dram = nc.dram_tensor("inv_in", [P, NT_ROW], f32, kind="ExternalInput")
    ident_dram = nc.dram_tensor("ident_f", [P, P], f32, kind="ExternalInput")
    bigI_dram = nc.dram_tensor("bigI", [P, P], f32, kind="ExternalInput")
    acc_dram = nc.dram_tensor("acc", [P, MROWS], bf16, kind="ExternalOutput")
    pos_dram = nc.dram_tensor("pos", [P, MB], f32, kind="ExternalOutput")

    with tile.TileContext(nc) as tc, ExitStack() as ctx:
        singles = ctx.enter_context(tc.tile_pool(name="singles", bufs=1))
        big = ctx.enter_context(tc.tile_pool(name="big", bufs=1))
        tmp_pool = ctx.enter_context(tc.tile_pool(name="tmp_pool", bufs=4))
        scr_pool = ctx.enter_context(tc.tile_pool(name="scr_pool", bufs=2))
        dram = ctx.enter_context(
            tc.tile_pool(name="dram", bufs=1, space=bass.MemorySpace.DRAM)
        )
        psum = ctx.enter_context(
            tc.tile_pool(name="psum", bufs=3, space=bass.MemorySpace.PSUM)
        )

        # --- constants / small inputs ---
        ident_f = singles.tile([P, P], f32)
        nc.sync.dma_start(out=ident_f, in_=ident_dram.ap())
        bigI = singles.tile([P, P], f32)
        nc.sync.dma_start(out=bigI, in_=bigI_dram.ap())
        inv = singles.tile([P, NT_ROW], f32)
        nc.sync.dma_start(out=inv, in_=inv_dram.ap())

        # --- persistent buffers ---
        zT0 = big.tile([P, R], bf16)            # [d 0:128, row]
        zT1 = big.tile([P, R], bf16)            # [d 128:256, row]
        zT = [zT0, zT1]
        acc = singles.tile([P, MROWS], bf16)    # running col-max, [j%128, i]
        pos_sb = singles.tile([P, MB], f32)
        znb_d = dram.tile([R, D], bf16)         # DRAM scratch for transpose

        nc.vector.memset(acc, -BIG)

        def preprocess(g):
            rs = slice(g * MROWS, (g + 1) * MROWS)
            # cast-DMA f32 -> bf16 DRAM->DRAM (SWDGE; big linear descriptors,
            # no compute engines, no SBUF staging)
            nc.gpsimd.dma_start(out=znb_d[rs, :], in_=z_dram.ap()[rs, :])
            # xbar-transpose the bf16 band into zT
            for c in range(KC):
                nc.sync.dma_start(
                    out=zT[c][:, rs],
                    in_=znb_d[rs, c * P : (c + 1) * P],
                    transpose=True,
                )

        def cell(jc):
            o = jc * P
            pp = psum.tile([P, MROWS], f32, name="pp")
            for c in range(KC):
                for u in range(MROWS // 512):
                    nc.tensor.matmul(
                        pp[:, u * 512 : (u + 1) * 512],
                        zT[c][:, o : o + P],
                        zT[c][:, u * 512 : (u + 1) * 512],
                        start=(c == 0),
                        stop=(c == KC - 1),
                    )
            if jc < MB:
                # self-similarity diagonal: i == j at free offset 128*jc
                nc.vector.tensor_sub(
                    pp[:, o : o + P], pp[:, o : o + P], bigI
                )
            if 4 * MB <= jc < 5 * MB:
                # positives: j == i + B diag at free offset 128*(jc-32)
                t = jc - 4 * MB
                scr = scr_pool.tile([P, P], f32, name="scr")
                nc.vector.tensor_mul(scr, pp[:, t * P : (t + 1) * P], ident_f)
                nc.vector.reduce_sum(
                    out=pos_sb[:, t : t + 1], in_=scr, axis=X
                )
            # evacuate with fused column scale inv[j] (per-partition AP)
            tmp = tmp_pool.tile([P, MROWS], bf16, name="tmp")
            if jc in DVE_CELLS:
                nc.vector.tensor_scalar_mul(tmp, pp[:], inv[:, jc : jc + 1])
            else:
                nc.scalar.mul(tmp, pp[:], inv[:, jc : jc + 1])
            nc.vector.tensor_max(acc, acc, tmp)

        # all preprocessing upfront: casts flow back-to-back on the gpsimd
        # queue, transposes trail each cast on the sync queue; cells then
        # gate on their band's transpose via data deps
        for g in range(CH):
            preprocess(g)
        for jc in range(NC_CELL):
            cell(jc)

        nc.sync.dma_start(out=acc_dram.ap(), in_=acc[:])
        nc.sync.dma_start(out=pos_dram.ap(), in_=pos_sb[:])

    nc.compile()
    return nc


def _get_nc():
    if "nc" not in _CACHE:
        _CACHE["nc"] = _build_nc()
    return _CACHE["nc"]


def _finish(inv64: np.ndarray, accs, poss) -> np.ndarray:
    """Host epilogue in f64: final 128-way max, exact row norms, mean/LSE."""
    rm = np.concatenate([np.asarray(a, dtype=np.float64).max(axis=0) for a in accs])
    pos_raw = np.concatenate(
        [np.asarray(p, dtype=np.float64).T.reshape(-1) for p in poss]
    )
    g = np.arange(R)
    negmax = rm * inv64 / TEMPERATURE
    pos = pos_raw * inv64 * inv64[(g + B) % R] / TEMPERATURE
    m = pos.max()
    lse = np.log(np.exp(pos - m).sum()) + m
    return np.array(negmax.mean() - lse, dtype=np.float32)


def kernel(z_i: np.ndarray, z_j: np.ndarray, _collect=None, _run_kwargs=None) -> np.ndarray:
    from concourse.bass_utils import run_bass_kernel_spmd

    z_full = np.concatenate(
        [np.asarray(z_i, np.float32), np.asarray(z_j, np.float32)], axis=0
    )
    inv64 = 1.0 / np.maximum(np.linalg.norm(z_full.astype(np.float64), axis=1), 1e-12)
    inv32 = inv64.astype(np.float32)
    consts = _host_constants()
    in_maps = [
        {
            "z": np.ascontiguousarray(np.roll(z_full, -k * MROWS, axis=0)),
            "inv_in": np.ascontiguousarray(
                np.roll(inv32, -k * MROWS).reshape(NT_ROW, P).T
            ),
            **consts,
        }
        for k in range(NCORES)
    ]
    nc = _get_nc()
    res = run_bass_kernel_spmd(
        nc, in_maps, core_ids=list(range(NCORES)), **(_run_kwargs or {})
    )
    if _collect is not None:
        _collect.append(res)
    accs = [r["acc"] for r in res.results]
    poss = [r["pos"] for r in res.results]
    return _finish(inv64, accs, poss)



# revision 2
# speedup vs baseline: 1.6565x; 1.6565x over previous
"""Contrastive loss kernel for Trainium2, 8 NeuronCores (SPMD).

Math (matches the reference):
    z = concat(normalize(z_i), normalize(z_j))        # (2B, D) = (8192, 256)
    sim = (z @ z.T) / T
    positives[g] = sim[g, (g+B) mod 2B]               # (2B,)
    neg_max[g] = max_{j != g} sim[g, j]
    loss = mean(neg_max) - logsumexp(positives)       # scalar

v8 design (symmetric half-Gram + exp-space epilogue):
  Host pre-normalizes z (f64 norms), rolls per core, and ships the bf16
  TRANSPOSED operand zT[d, row] directly -- no on-device cast/transpose.

  Gram symmetry: core k computes only stationary j-chunks 0..39 (local bands
  0..4 = own band + next 4) against its own 1024 moving rows: 40 cells of
  [128 j, 1024 i] = 62.5% of the full Gram. Each computed cell credits its
  MOVING rows via an elementwise max-accumulate (i-side); cells whose j-rows
  are not credited elsewhere (local bands 1..3, cells 8..31) also need a
  row-collapse over the free axis (j-side). Band 0 (cells 0..7) contains
  both orderings internally; band 4 (cells 32..39) is computed by two cores
  (antipodal class), so its j-side is the partner's i-side.

  Epilogue engine split (measured: DVE f32-psum ops ~1.2us/cell, bf16
  elementwise 0.68us (2x mode), reduces always ~1.2us; ACT copy 1.12us,
  +0.32us for its sum-accumulator):
    cells 0..7  : DVE sub BIG on self-diag chunk, DVE max-acc psum->acc_raw.
    cells 8..31 : ACT activation Exp(80*sim) -> bf16 expcp + accumulator
                  sum_i exp = soft row-max (bias T2*ln(#near-max) ~ 0.008,
                  validated 1.8e-3 final rel err in fp-accurate emulation);
                  DVE bf16 max-acc expcp -> acc_exp (exact: max of exps).
                  A few cells instead take the exact DVE route (reduce_max +
                  psum max-acc) to balance ACT vs DVE queues.
    cells 32..39: ACT Exp (no accum) + DVE max-acc; positives extracted from
                  the expcp diagonal (ident mul + reduce_sum).
  Host (f64): 128-way partition maxes, T2*ln() back to sim units, cross-core
  row assembly, mean/LSE.
"""

import numpy as np

TEMPERATURE = 0.1
B, D = 4096, 256
R = 2 * B                # 8192 total rows
NCORES = 8
MROWS = R // NCORES      # 1024 rows per core
P = 128                  # SBUF partitions
NCELL = 40               # stationary j-chunks per core (5 bands)
NJROW = NCELL * P        # 5120 stationary rows shipped
KC = D // P              # 2 contraction chunks of 128
BIG = 30000.0            # diag mask subtrahend
T2 = 1.0 / 80.0          # exp-space temperature (scale=80)
DVE_J_CELLS = frozenset((30, 31))  # j-cells on the exact DVE route (balance)

_CACHE = {}


def _host_constants():
    import ml_dtypes
    ident_bf = np.eye(P, dtype=np.float32).astype(ml_dtypes.bfloat16)
    bigI = (np.eye(P) * BIG).astype(np.float32)
    return {"ident_bf": ident_bf, "bigI": bigI}


def _build_nc():
    from contextlib import ExitStack

    import concourse.bass as bass
    import concourse.mybir as mybir
    import concourse.tile as tile
    from concourse import bacc

    f32 = mybir.dt.float32
    bf16 = mybir.dt.bfloat16
    X = mybir.AxisListType.X
    EXP = mybir.ActivationFunctionType.Exp

    nc = bacc.Bacc(
        "TRN2",
        target_bir_lowering=False,
        debug=False,
        enable_asserts=False,
        num_devices=NCORES,
    )

    zt0_dram = nc.dram_tensor("zt0", [P, NJROW], bf16, kind="ExternalInput")
    zt1_dram = nc.dram_tensor("zt1", [P, NJROW], bf16, kind="ExternalInput")
    ident_dram = nc.dram_tensor("ident_bf", [P, P], bf16, kind="ExternalInput")
    bigI_dram = nc.dram_tensor("bigI", [P, P], f32, kind="ExternalInput")
    accr_dram = nc.dram_tensor("acc_raw", [P, MROWS], bf16, kind="ExternalOutput")
    acce_dram = nc.dram_tensor("acc_exp", [P, MROWS], bf16, kind="ExternalOutput")
    esum_dram = nc.dram_tensor("expsum", [P, NCELL], f32, kind="ExternalOutput")
    rmax_dram = nc.dram_tensor("rmax", [P, NCELL], f32, kind="ExternalOutput")
    pose_dram = nc.dram_tensor("pos_exp", [P, MROWS // P], f32, kind="ExternalOutput")

    with tile.TileContext(nc) as tc, ExitStack() as ctx:
        singles = ctx.enter_context(tc.tile_pool(name="singles", bufs=1))
        exp_pool = ctx.enter_context(tc.tile_pool(name="exp_pool", bufs=4))
        scr_pool = ctx.enter_context(tc.tile_pool(name="scr_pool", bufs=2))
        psum = ctx.enter_context(
            tc.tile_pool(name="psum", bufs=4, space=bass.MemorySpace.PSUM)
        )

        # --- persistent buffers ---
        zt0 = singles.tile([P, NJROW], bf16)
        zt1 = singles.tile([P, NJROW], bf16)
        zT = [zt0, zt1]
        ident_bf = singles.tile([P, P], bf16)
        bigI = singles.tile([P, P], f32)
        acc_raw = singles.tile([P, MROWS], bf16)
        acc_exp = singles.tile([P, MROWS], bf16)
        expsum = singles.tile([P, NCELL], f32)
        rmax = singles.tile([P, NCELL], f32)
        pos_exp = singles.tile([P, MROWS // P], f32)

        # --- input DMA: moving band (first 1024 cols of both KC chunks) first
        # so matmuls can start early; two queues in parallel ---
        nc.sync.dma_start(out=zt0[:, :MROWS], in_=zt0_dram.ap()[:, :MROWS])
        nc.gpsimd.dma_start(out=zt1[:, :MROWS], in_=zt1_dram.ap()[:, :MROWS])
        nc.sync.dma_start(out=zt0[:, MROWS:], in_=zt0_dram.ap()[:, MROWS:])
        nc.gpsimd.dma_start(out=zt1[:, MROWS:], in_=zt1_dram.ap()[:, MROWS:])
        nc.sync.dma_start(out=ident_bf, in_=ident_dram.ap())
        nc.sync.dma_start(out=bigI, in_=bigI_dram.ap())

        nc.vector.memset(acc_raw, -BIG)
        nc.vector.memset(acc_exp, 0.0)
        nc.vector.memset(expsum, 0.0)
        nc.vector.memset(rmax, -BIG)
        nc.vector.memset(pos_exp, 1.0)

        def cell(s):
            o = s * P
            pp = psum.tile([P, MROWS], f32, name="pp")
            for c in range(KC):
                for u in range(MROWS // 512):
                    nc.tensor.matmul(
                        pp[:, u * 512:(u + 1) * 512],
                        zT[c][:, o:o + P],
                        zT[c][:, u * 512:(u + 1) * 512],
                        start=(c == 0),
                        stop=(c == KC - 1),
                    )
            if s < 8:
                # self-similarity diagonal at free offset 128*s
                nc.vector.tensor_sub(pp[:, o:o + P], pp[:, o:o + P], bigI)
                nc.vector.tensor_max(acc_raw, acc_raw, pp[:])
            elif s < 32:
                if s in DVE_J_CELLS:
                    nc.vector.reduce_max(out=rmax[:, s:s + 1], in_=pp[:], axis=X)
                    nc.vector.tensor_max(acc_raw, acc_raw, pp[:])
                else:
                    ec = exp_pool.tile([P, MROWS], bf16, name="ec")
                    nc.scalar.activation(
                        ec, pp[:], EXP, scale=1.0 / T2,
                        accum_out=expsum[:, s:s + 1],
                    )
                    nc.vector.tensor_max(acc_exp, acc_exp, ec)
            else:
                t = s - 32
                ec = exp_pool.tile([P, MROWS], bf16, name="ec")
                nc.scalar.activation(ec, pp[:], EXP, scale=1.0 / T2)
                nc.vector.tensor_max(acc_exp, acc_exp, ec)
                # positives: diagonal of chunk t (j = i + B in rolled coords)
                scr = scr_pool.tile([P, P], bf16, name="scr")
                nc.vector.tensor_mul(scr, ec[:, t * P:(t + 1) * P], ident_bf)
                nc.vector.reduce_sum(out=pos_exp[:, t:t + 1], in_=scr, axis=X)

        for s in range(NCELL):
            cell(s)

        nc.sync.dma_start(out=accr_dram.ap(), in_=acc_raw[:])
        nc.sync.dma_start(out=acce_dram.ap(), in_=acc_exp[:])
        nc.sync.dma_start(out=esum_dram.ap(), in_=expsum[:])
        nc.sync.dma_start(out=rmax_dram.ap(), in_=rmax[:])
        nc.sync.dma_start(out=pose_dram.ap(), in_=pos_exp[:])

    nc.compile()
    return nc


def _get_nc():
    if "nc" not in _CACHE:
        _CACHE["nc"] = _build_nc()
    return _CACHE["nc"]


def _finish(results) -> np.ndarray:
    """Host epilogue in f64: partition maxes, ln back to sim units, assembly."""
    negmax = np.full(R, -np.inf)
    pos = np.empty(R)
    for k in range(NCORES):
        r = results[k]
        roll = (np.arange(R) + k * MROWS) % R  # local index -> global row
        acc_raw = np.asarray(r["acc_raw"], dtype=np.float64)
        acc_exp = np.asarray(r["acc_exp"], dtype=np.float64)
        expsum = np.asarray(r["expsum"], dtype=np.float64)
        rmax = np.asarray(r["rmax"], dtype=np.float64)
        pos_exp = np.asarray(r["pos_exp"], dtype=np.float64)

        # own rows (local 0..1023): i-side credit
        own = np.maximum(
            acc_raw.max(axis=0),
            T2 * np.log(np.maximum(acc_exp.max(axis=0), 1e-300)),
        )
        g = roll[:MROWS]
        np.maximum.at(negmax, g, own)

        # j-side credit for cells 8..31 (local rows 1024..4095)
        jvals = np.where(
            np.isin(np.arange(NCELL), list(DVE_J_CELLS))[None, :],
            rmax,
            T2 * np.log(np.maximum(expsum, 1e-300)),
        )  # [128, 40]; only cols 8..31 meaningful
        lrows = np.arange(8 * P, 32 * P)
        np.maximum.at(negmax, roll[lrows], jvals[:, 8:32].T.reshape(-1))

        # positives for own rows: pos[g] = sim[g, g+B]
        pv = T2 * np.log(np.maximum(pos_exp.T.reshape(-1), 1e-300))  # local 0..1023
        pos[roll[:MROWS]] = pv

    negmax /= TEMPERATURE
    pos /= TEMPERATURE
    m = pos.max()
    lse = np.log(np.exp(pos - m).sum()) + m
    return np.array(negmax.mean() - lse, dtype=np.float32)


def kernel(z_i: np.ndarray, z_j: np.ndarray, _collect=None, _run_kwargs=None) -> np.ndarray:
    import ml_dtypes
    from concourse.bass_utils import run_bass_kernel_spmd

    z = np.concatenate(
        [np.asarray(z_i, np.float32), np.asarray(z_j, np.float32)], axis=0
    )
    inv = 1.0 / np.maximum(np.linalg.norm(z.astype(np.float64), axis=1), 1e-12)
    zhat = (z * inv[:, None].astype(np.float32)).astype(np.float32)
    zbf = zhat.astype(ml_dtypes.bfloat16)  # [R, D]
    consts = _host_constants()
    in_maps = []
    for k in range(NCORES):
        zk = np.roll(zbf, -k * MROWS, axis=0)[:NJROW]  # [5120, 256]
        zkT = np.ascontiguousarray(zk.T)               # [256, 5120]
        in_maps.append({
            "zt0": zkT[:P],
            "zt1": zkT[P:],
            **consts,
        })
    nc = _get_nc()
    res = run_bass_kernel_spmd(
        nc, in_maps, core_ids=list(range(NCORES)), **(_run_kwargs or {})
    )
    if _collect is not None:
        _collect.append(res)
    return _finish(res.results)
